# revision 21
# baseline (speedup 1.0000x reference)
"""Multi-head self-attention TRN2 Bass kernel (8 NeuronCores).

Sharding: core c handles batch b = c // 4 and head group g = c % 4
(heads 4g..4g+3).  Data parallel over B, tensor parallel over heads:
each core projects q/k/v for its 4 heads, runs attention, and computes
a partial output projection over its 256 ctx dims.  Host sums the 4
partials per batch (row-parallel unshard) and adds out_b.

Per-core layout tricks:
- scores are computed transposed (keys on partitions, queries on the
  free dim) so that exp(scores^T) feeds the PV matmul directly as the
  moving operand -- no transposes anywhere in the kernel;
- the two heads of a pair occupy the two 64-row halves of the PE array
  (tile_position row tiling), so their Dh=64-contraction QK matmuls run
  concurrently;
- the softmax denominator falls out of a 65th "ones" column appended to
  V, and the normalization is applied to ctx^T with a gpsimd
  partition-broadcast of 1/denom plus one vector multiply;
- all matmul operands are float32r (full-rate PE streaming, ~1e-4
  mantissa precision); set MM_DT = float32 for full fp32 at 4x PE cost.
"""

import numpy as np

import concourse.mybir as mybir
import concourse.tile as tile
from concourse import bacc
from concourse import bass_utils

F32 = mybir.dt.float32

B = 2
T = 2048
D = 1024
H = 16
DH = 64
N_CORES = 8
G = 4  # head groups
HPC = 4  # heads per core
EQK = 512  # q rows + k rows per core
EV = 256  # v rows per core
SCALE = DH ** -0.5

# float32r streams through the PE at 1 cycle/row (vs 4 for float32) at
# reduced mantissa precision (~1e-4 rel).  Flip to mybir.dt.float32 for
# full precision at 4x the PE time.
MM_DT = mybir.dt.float32r



TT = T // 512  # 4 q-tiles of 512
TB = T // 128  # 16 t-blocks of 128
DC = D // 128  # 8 d-chunks of 128


def build_nc(repeats=1):
    nc = bacc.Bacc("TRN2", target_bir_lowering=False, debug=False,
                   num_devices=N_CORES)

    xT = nc.dram_tensor("xT", [D, T], MM_DT, kind="ExternalInput").ap()
    wqkT = nc.dram_tensor("wqkT", [D, EQK], MM_DT, kind="ExternalInput").ap()
    wvT = nc.dram_tensor("wvT", [D, EV], MM_DT, kind="ExternalInput").ap()
    bqk = nc.dram_tensor("bqk", [128, 4], F32, kind="ExternalInput").ap()
    bv = nc.dram_tensor("bv", [1, EV], MM_DT, kind="ExternalInput").ap()
    onesd = nc.dram_tensor("onesd", [1, 128], MM_DT, kind="ExternalInput").ap()
    onescol = nc.dram_tensor("onescol", [128, HPC], MM_DT, kind="ExternalInput").ap()
    woT = nc.dram_tensor("woT", [EV, D], MM_DT, kind="ExternalInput").ap()
    y = nc.dram_tensor("y", [T, D], F32, kind="ExternalOutput").ap()

    with tile.TileContext(nc) as tc:
        for rep in range(repeats):
            _emit(tc, nc, xT, wqkT, wvT, bqk, bv, onesd, onescol, woT, y,
                  suffix=f"_r{rep}" if repeats > 1 else "")

    nc.compile()
    return nc


def _emit(tc, nc, xT, wqkT, wvT, bqk, bv, onesd, onescol, woT, y, suffix=""):
    import contextlib
    s = suffix
    ctx = contextlib.ExitStack()
    with ctx:
        consts = ctx.enter_context(tc.tile_pool(name=f"consts{s}", bufs=1))
        expp = ctx.enter_context(tc.tile_pool(name=f"expp{s}", bufs=4))
        smalls = ctx.enter_context(tc.tile_pool(name=f"smalls{s}", bufs=2))
        ypool = ctx.enter_context(tc.tile_pool(name=f"ypool{s}", bufs=2))
        ps_mm = ctx.enter_context(tc.tile_pool(name=f"ps_mm{s}", bufs=2, space="PSUM"))
        ps_s = ctx.enter_context(tc.tile_pool(name=f"ps_s{s}", bufs=2, space="PSUM"))
        ps_ctx = ctx.enter_context(tc.tile_pool(name=f"ps_ctx{s}", bufs=2, space="PSUM"))

        # ---- load inputs (weights/consts first: the first projection
        #      matmul needs wqk + xt[0], so x streams in behind them) ----
        wqk = consts.tile([128, DC, EQK], MM_DT, tag="wqk")
        nc.sync.dma_start(out=wqk, in_=wqkT.rearrange("(c p) e -> p c e", p=128))
        bqk_sb = consts.tile([128, 4], F32, tag="bqk")
        nc.sync.dma_start(out=bqk_sb, in_=bqk)
        bv_sb = consts.tile([1, EV], MM_DT, tag="bv")
        nc.sync.dma_start(out=bv_sb, in_=bv)
        ones = consts.tile([1, 128], MM_DT, tag="ones")
        nc.sync.dma_start(out=ones, in_=onesd)
        wv = consts.tile([128, DC, EV], MM_DT, tag="wv")
        xt = [consts.tile([128, T], MM_DT, tag=f"xt{i}", name=f"xt{i}")
              for i in range(DC)]
        for h in range(2):
            lo, hi = h * T // 2, (h + 1) * T // 2
            for i in range(DC):
                nc.sync.dma_start(out=xt[i][:, lo:hi],
                                  in_=xT[i * 128:(i + 1) * 128, lo:hi])
            if h == 0:
                nc.sync.dma_start(
                    out=wv, in_=wvT.rearrange("(c p) e -> p c e", p=128))
        wo = consts.tile([128, 2, D], MM_DT, tag="wo")
        nc.sync.dma_start(out=wo, in_=woT.rearrange("(c p) e -> p c e", p=128))

        # ---- q/k projection (transposed layout: e on partitions, t free) ----
        # qk[eb][tt]: e-block eb (0-1: q heads 01/23, 2-3: k heads 01/23)
        qk = [[consts.tile([128, 512], MM_DT, tag=f"qk{eb}_{tt}", name=f"qk{eb}_{tt}")
               for tt in range(TT)] for eb in range(4)]

        def emit_qk_proj(eb, tts=None):
            for tt in (range(TT) if tts is None else tts):
                ps = ps_mm.tile([128, 512], F32, tag="ps_mm", name="ps")
                for dc in range(DC):
                    nc.tensor.matmul(
                        ps,
                        (wqk[:, dc, eb * 128:(eb + 1) * 128]),
                        (xt[dc][:, tt * 512:(tt + 1) * 512]),
                        start=(dc == 0), stop=(dc == DC - 1))
                # add bias (per-partition) while evacuating psum
                nc.vector.tensor_scalar_add(qk[eb][tt], ps, bqk_sb[:, eb:eb + 1])

        # ---- v projection (natural layout: t on partitions, head dims free,
        #      65th column per head = 1.0 for the softmax denominator) ----
        v = [consts.tile([128, HPC, DH + 1], MM_DT, tag=f"v{tb}", name=f"v{tb}")
             for tb in range(TB)]

        def emit_v_proj(tb):
            nc.sync.dma_start(
                out=v[tb][:, :, DH:DH + 1],
                in_=onescol.rearrange("p (h o) -> p h o", o=1))
            ps = ps_mm.tile([128, 512], F32, tag="ps_mm", name="ps")
            psv = ps[:, 0:EV]
            for dc in range(DC):
                nc.tensor.matmul(
                    psv,
                    (xt[dc][:, tb * 128:(tb + 1) * 128]),
                    (wv[:, dc, :]),
                    start=(dc == 0), stop=False)
            nc.tensor.matmul(psv, (ones), (bv_sb), start=False, stop=True)
            nc.vector.tensor_copy(
                v[tb][:, :, 0:DH],
                psv.rearrange("p (h d) -> p h d", h=HPC))

        # ---- attention ----
        # ctxc[qt]: ctx^T, 128 rows = 2 chunks x (2 heads x 64 dims), per q-tile
        ctxc = [consts.tile([128, 2, 512], MM_DT, tag=f"ctx{qt}", name=f"ctx{qt}")
                for qt in range(TT)]

        def emit_attn_unit(qt, hp, pre_kc=None):
            # head pair (2*hp, 2*hp+1) on array row halves
            qeb, keb = hp, 2 + hp
            pctx2 = [ps_ctx.tile([65, 512], F32, tag=f"ps_ctx{i}",
                                 name=f"pctx{i}", bufs=1) for i in range(2)]
            for kc in range(TB):
                if pre_kc is not None:
                    pre_kc(kc)
                # both heads' scores^T chunks concurrently via row tiling
                pss = ps_s.tile([128, 1024], F32, tag="ps_s", name="pss")
                for half in range(2):
                    po = half * 64
                    nc.tensor.matmul(
                        pss[:, half * 512:(half + 1) * 512],
                        (qk[keb][kc // 4][po:po + 64,
                                          (kc % 4) * 128:(kc % 4 + 1) * 128]),
                        (qk[qeb][qt][po:po + 64, :]),
                        start=True, stop=True,
                        tile_position=(po, 0))
                et = expp.tile([128, 1024], MM_DT, tag="exp", name="et")
                nc.scalar.activation(out=et, in_=pss,
                                     func=mybir.ActivationFunctionType.Exp,
                                     scale=SCALE)
                for half in range(2):
                    nc.tensor.matmul(
                        pctx2[half],
                        (v[kc][:, 2 * hp + half, :]),
                        (et[:, half * 512:(half + 1) * 512]),
                        start=(kc == 0), stop=(kc == TB - 1))
            # normalize: reciprocal of denom row, gpsimd-broadcast, multiply
            for half in range(2):
                po = half * 64
                dr = smalls.tile([1, 512], F32, tag="dr", name="dr")
                nc.vector.tensor_copy(dr, pctx2[half][64:65, :])
                rb1 = smalls.tile([1, 512], F32, tag="rb1", name="rb1")
                nc.vector.reciprocal(out=rb1, in_=dr)
                rbb = smalls.tile([64, 512], F32, tag="rbb", name="rbb")
                nc.gpsimd.partition_broadcast(rbb, rb1)
                nc.vector.tensor_mul(
                    ctxc[qt][po:po + 64, hp, :], pctx2[half][0:64, :], rbb)

        def emit_out_proj(qt):
            # partial out-proj for this q-tile: y[t, e] = sum_d ctxT[d,t]*woT[d,e]
            for ti in range(4):
                tb = qt * 4 + ti
                ysb = ypool.tile([128, D], F32, tag="y", name="ysb")
                for et in range(2):
                    ps = ps_mm.tile([128, 512], F32, tag="ps_mm", name="ps")
                    for cc in range(2):
                        nc.tensor.matmul(
                            ps,
                            (ctxc[qt][:, cc, ti * 128:(ti + 1) * 128]),
                            (wo[:, cc, et * 512:(et + 1) * 512]),
                            start=(cc == 0), stop=(cc == 1))
                    nc.vector.tensor_copy(ysb[:, et * 512:(et + 1) * 512], ps)
                nc.sync.dma_start(out=y[tb * 128:(tb + 1) * 128, :], in_=ysb)

        # Emission order tuned for overlap: head-pair 0's four units only
        # need the k(h01)/q(h01) projections, so they run first while the
        # remaining projections weave into their PE slack; v-proj units are
        # woven into the first unit's chunk loop (v[kc] lands just before
        # the PV that consumes it).
        emit_qk_proj(2)
        emit_qk_proj(0, tts=[0])
        emit_attn_unit(0, 0, pre_kc=emit_v_proj)
        emit_qk_proj(0, tts=[1])
        emit_attn_unit(1, 0)
        emit_qk_proj(0, tts=[2, 3])
        emit_qk_proj(3)
        emit_attn_unit(2, 0)
        emit_attn_unit(3, 0)
        emit_qk_proj(1)
        emit_attn_unit(0, 1)
        emit_out_proj(0)
        emit_attn_unit(1, 1)
        emit_out_proj(1)
        emit_attn_unit(2, 1)
        emit_attn_unit(3, 1)
        emit_out_proj(2)
        emit_out_proj(3)


def make_in_maps(x, qkv_w, qkv_b, out_w):
    """Slice + pre-transpose full inputs into per-core input maps."""
    x = np.asarray(x, dtype=np.float32)
    qkv_w = np.asarray(qkv_w, dtype=np.float32)
    qkv_b = np.asarray(qkv_b, dtype=np.float32)
    out_w = np.asarray(out_w, dtype=np.float32)
    in_maps = []
    for c in range(N_CORES):
        b, g = c // G, c % G
        r0 = g * 256
        wq = qkv_w[r0:r0 + 256]
        wk = qkv_w[D + r0:D + r0 + 256]
        wv_ = qkv_w[2 * D + r0:2 * D + r0 + 256]
        bq = qkv_b[r0:r0 + 256]
        bk = qkv_b[D + r0:D + r0 + 256]
        bv_ = qkv_b[2 * D + r0:2 * D + r0 + 256]
        in_maps.append({
            "xT": np.ascontiguousarray(x[b].T),
            "wqkT": np.ascontiguousarray(np.concatenate([wq, wk], 0).T),
            "wvT": np.ascontiguousarray(wv_.T),
            "bqk": np.ascontiguousarray(
                np.concatenate([bq, bk]).reshape(4, 128).T),
            "bv": np.ascontiguousarray(bv_.reshape(1, EV)),
            "onesd": np.ones((1, 128), np.float32),
            "onescol": np.ones((128, HPC), np.float32),
            "woT": np.ascontiguousarray(out_w[:, r0:r0 + 256].T),
        })
    return in_maps


def unshard(results, out_b):
    """Sum the 4 per-core partials per batch and add bias."""
    out = np.empty((B, T, D), dtype=np.float32)
    for b in range(B):
        acc = results[b * G]["y"].astype(np.float32).copy()
        for g in range(1, G):
            acc += results[b * G + g]["y"]
        out[b] = acc + np.asarray(out_b, dtype=np.float32)[None, :]
    return out


_NC = None


def kernel(x, qkv_w, qkv_b, out_w, out_b):
    global _NC
    if _NC is None:
        _NC = build_nc()
    in_maps = make_in_maps(x, qkv_w, qkv_b, out_w)
    res = bass_utils.run_bass_kernel_spmd(_NC, in_maps, list(range(N_CORES)))
    return unshard(res.results, out_b)
